# revision 40
# baseline (speedup 1.0000x reference)
"""Trainium2 Bass kernel for nn_Encoder_Flows (3-layer dense GCN, linear).

The reference network has no nonlinearity, so per graph (A = flows [N,N]):
    out = ^A^3 (A @ W123) + bias-terms,   W123 = W1@W2@W3  (host-precomputed)
    ^A = D^-1/2 A^T D^-1/2,  deg[c] = sum_r A[r,c]
Bias terms are rank-1 (zero for the graded inputs; added on host if present).

A is centered on host: At~ = A - 0.5 (fp8 e4m3), so A = At~ + 0.5*ones.
The 0.5*ones*ones^T rank-1 part of every product collapses to a per-feature
constant (0.5 * colsum of the multiplicand) that is fused into the PSUM-drain
op; centering also kills the systematic fp8 quantization error that raw-A
fp8 matmuls would amplify by deg~1024.

All big matmuls keep A as the fp8 DoubleRow MOVING operand (2 elem/cycle;
the PE's stationary-load port is the bottleneck when A is stationary), with
small stationary tiles (W123 / msg k-tile pairs), so the whole chain is
computed feature-major:
  deg    : ones-stationary DR matmul over An chunks -> deg replicated [128,N]
  scales : sqrt (scalar) -> reciprocal/square (DVE, f16 replicated rows)
  pass0  : P0 = (32 W123)^T At~^T            (At~ chunks moving)
  pass k : Pk = msg_k^T At~                  (An chunks moving)
  drain  : msgT_{k+1} = (Pk + 0.5 sig_k[d]) * scale_rep  -- one DVE op per
           512-col PSUM piece, sig accumulated by the same op (accum_out)
  msgq   : 16 PE transposes per pass give the node-major fp8 stationary
           tiles for the next pass
  out    : (P3 + 0.5 sig3) * dinv/32, written feature-major [g, D, N]; the
           host transposes back to [g, N, D].
"""

import sys
from contextlib import ExitStack

import numpy as np

for _p in ("/opt/trn_rl_repo", "/opt/pypackages"):
    if _p not in sys.path:
        sys.path.append(_p)

import ml_dtypes

B, N, P = 16, 2048, 128
NB = N // P          # 16 row/col blocks
NQ = NB // 2         # 8 DoubleRow k-tile pairs
NCORES = 8
GPC = B // NCORES    # graphs per core
D = 128              # folded feature width (W123 columns)
CH = 512             # DMA / psum-piece column chunk
NCH = N // CH        # 4

_COMPILED = {}


def _build():
    import concourse.mybir as mybir
    import concourse.tile as tile
    from concourse import bacc
    from concourse.masks import make_identity

    f32 = mybir.dt.float32
    f16 = mybir.dt.float16
    fp8 = mybir.dt.float8e4
    DR = mybir.MatmulPerfMode.DoubleRow
    ADD = mybir.AluOpType.add
    MUL = mybir.AluOpType.mult
    X = mybir.AxisListType.X

    nc = bacc.Bacc("TRN2", target_bir_lowering=False)
    An_d = nc.declare_dram_parameter("An", [GPC, N, N], fp8, isOutput=False)
    At_d = nc.declare_dram_parameter("At", [GPC, N, N], fp8, isOutput=False)
    W_d = nc.declare_dram_parameter("W32", [N, D], fp8, isOutput=False)
    cwh_d = nc.declare_dram_parameter("cwh", [P, 1], f32, isOutput=False)
    out_d = nc.declare_dram_parameter("out", [GPC, D, N], f16, isOutput=True)

    with tile.TileContext(nc) as tc, ExitStack() as ctx:
        wpool = ctx.enter_context(tc.tile_pool(name="wpool", bufs=1))
        apool = ctx.enter_context(tc.tile_pool(name="apool", bufs=2))
        tpool = ctx.enter_context(tc.tile_pool(name="tpool", bufs=2))
        mtp = ctx.enter_context(tc.tile_pool(name="mtp", bufs=3))
        mqp = ctx.enter_context(tc.tile_pool(name="mqp", bufs=2))
        svec = ctx.enter_context(tc.tile_pool(name="svec", bufs=2))
        slim = ctx.enter_context(tc.tile_pool(name="slim", bufs=4))
        ogp = ctx.enter_context(tc.tile_pool(name="ogp", bufs=4))
        ppool = ctx.enter_context(tc.tile_pool(name="ppool", bufs=4, space="PSUM"))
        scr = ctx.enter_context(tc.tile_pool(name="scr", bufs=4, space="PSUM"))

        # --- replicated constants (DMAs deferred until after An0) ---
        W32_sb = wpool.tile([P, NB, D], fp8)
        cwh = wpool.tile([P, 1], f32)
        onesf8 = wpool.tile([P, 2, P], fp8)
        nc.any.memset(onesf8[:], 1.0)
        idf16 = wpool.tile([P, P], f16)
        make_identity(nc, idf16[:])
        c1024 = wpool.tile([P, 1], f32)
        nc.any.memset(c1024[:], 1024.0)

        st = [{} for _ in range(GPC)]   # per-graph tiles

        # ---- DMA + deg + pass0, chunk-interleaved ----------------------
        def emit_dma(g):
            # one tile per DoubleRow k-pair, full 2048B rows per descriptor;
            # An first so deg/dinv complete while At streams. Issued for both
            # graphs before any output DMA so the in-order sync engine never
            # stalls g1's inputs behind g0's outputs.
            s = st[g]
            s["An"] = [apool.tile([P, 2, N], fp8, tag=f"An{q}", name=f"An{g}_{q}") for q in range(NQ)]
            s["At"] = [tpool.tile([P, 2, N], fp8, tag=f"At{q}", name=f"At{g}_{q}") for q in range(NQ)]
            for q in range(NQ):
                nc.sync.dma_start(
                    s["An"][q][:],
                    An_d.ap()[g].rearrange("(rb p) c -> p rb c", p=P)[:, 2 * q:2 * q + 2, :])
            for q in range(NQ):
                nc.sync.dma_start(
                    s["At"][q][:],
                    At_d.ap()[g].rearrange("(fb p) m -> p fb m", p=P)[:, 2 * q:2 * q + 2, :])

        def emit_head(g):
            s = st[g]
            # deg: ones-stationary, An moving, streamed in DMA-arrival order
            dchunks = [scr.tile([P, CH], f32, tag="scr", name=f"deg{g}_{ch}") for ch in range(NCH)]
            for ch in range(NCH):
                for h in range(2):
                    for q in range(NQ):
                        nc.tensor.matmul(
                            dchunks[ch][:, h * 256:(h + 1) * 256],
                            onesf8[:],
                            s["An"][q][:, :, ch * CH + h * 256:ch * CH + (h + 1) * 256],
                            start=(q == 0), stop=(q == NQ - 1), perf_mode=DR)

            # replicated scale rows, per-chunk pipeline
            dinv = svec.tile([P, N], f32, tag="dinv", name=f"dinv{g}")
            rdeg = svec.tile([P, N], f16, tag="rdeg", name=f"rdeg{g}")
            dinvh = svec.tile([P, N], f16, tag="dinvh", name=f"dinvh{g}")
            for ch in range(NCH):
                sl = slice(ch * CH, (ch + 1) * CH)
                sdeg = slim.tile([P, CH], f32, tag="sdeg", name=f"sdeg{g}_{ch}")
                nc.scalar.activation(
                    sdeg[:], dchunks[ch][:],
                    mybir.ActivationFunctionType.Sqrt, bias=c1024[:])
                nc.vector.reciprocal_approx_fast(dinv[:, sl], sdeg[:])
                with nc.allow_low_precision(reason="f16 scale rows; 5e-4 rel ok vs 2e-2 tol"):
                    nc.vector.tensor_tensor(rdeg[:, sl], dinv[:, sl], dinv[:, sl], MUL)
                    nc.scalar.mul(dinvh[:, sl], dinv[:, sl], 1.0 / 32.0)

            # pass0: W32 stationary, At moving (streamed) -> P0 = Y0^T
            s["p0"] = [ppool.tile([P, CH], f32, tag="pp", name=f"pp0_{g}_{ch}") for ch in range(NCH)]
            for ch in range(NCH):
                for h in range(2):
                    for q in range(NQ):
                        nc.tensor.matmul(
                            s["p0"][ch][:, h * 256:(h + 1) * 256],
                            W32_sb[:, 2 * q:2 * q + 2, :],
                            s["At"][q][:, :, ch * CH + h * 256:ch * CH + (h + 1) * 256],
                            start=(q == 0), stop=(q == NQ - 1), perf_mode=DR)
            s["dinv"], s["rdeg"], s["dinvh"] = dinv, rdeg, dinvh

        # drain psum pieces -> msgT (f16) + sig half-colsum (no PE work)
        def emit_sst(g, k, pieces, scol, srep):
            s = st[g]
            msgT = mtp.tile([P, N], f16, tag="msgT", name=f"msgT{g}_{k}")
            sig = slim.tile([P, NCH], f32, tag="sig", name=f"sg{g}_{k}")
            for ch in range(NCH):
                nc.vector.scalar_tensor_tensor(
                    msgT[:, ch * CH:(ch + 1) * CH], pieces[ch][:], scol,
                    srep[:, ch * CH:(ch + 1) * CH], ADD, MUL,
                    accum_out=sig[:, ch:ch + 1])
            sigh = slim.tile([P, 1], f32, tag="sigh", name=f"sgh{g}_{k}")
            nc.vector.tensor_reduce(sigh[:], sig[:], X, ADD)
            nc.scalar.mul(sigh[:], sigh[:], 0.5)
            s["msgT"], s["sigh"] = msgT, sigh

        # PE transposes of msgT -> node-major fp8 stationary for next pass
        # (batched 4 per PSUM bank; one 512-wide cast-copy per batch,
        # alternating scalar/DVE)
        def emit_transp(g, k):
            s = st[g]
            msgq = mqp.tile([P, NB, D], fp8, tag="msgq", name=f"msgq{g}_{k}")
            for grp in range(NB // 4):
                tp = scr.tile([P, 4, P], f16, tag="scr", name=f"tp{g}_{k}_{grp}")
                for j in range(4):
                    mb = grp * 4 + j
                    nc.tensor.transpose(tp[:, j, :], s["msgT"][:, mb * P:(mb + 1) * P], idf16[:])
                if grp % 2 == 0:
                    nc.scalar.copy(msgq[:, grp * 4:grp * 4 + 4, :], tp[:])
                else:
                    nc.vector.tensor_copy(msgq[:, grp * 4:grp * 4 + 4, :], tp[:])
            s["msgq"] = msgq

        # pass k chunk: msgq stationary, An moving -> P feature-major piece
        def emit_pass(g, k):
            s = st[g]
            msgq = s["msgq"]
            pieces = []
            for ch in range(NCH):
                pp = ppool.tile([P, CH], f32, tag="pp", name=f"pp{k}_{g}_{ch}")
                for h in range(2):
                    for q in range(NQ):
                        nc.tensor.matmul(
                            pp[:, h * 256:(h + 1) * 256],
                            msgq[:, 2 * q:2 * q + 2, :],
                            s["An"][q][:, :, ch * CH + h * 256:ch * CH + (h + 1) * 256],
                            start=(q == 0), stop=(q == NQ - 1), perf_mode=DR)
                pieces.append(pp)
            return pieces

        def emit_out(g, pieces):
            s = st[g]
            for ch in range(NCH):
                og = ogp.tile([P, CH], f16, tag="og", name=f"og{g}_{ch}")
                with nc.allow_low_precision(reason="f16 output; 5e-4 rel ok vs 2e-2 tol"):
                    nc.vector.scalar_tensor_tensor(
                        og[:], pieces[ch][:], s["sigh"][:],
                        s["dinvh"][:, ch * CH:(ch + 1) * CH], ADD, MUL)
                nc.sync.dma_start(out_d.ap()[g][:, ch * CH:(ch + 1) * CH], og[:])

        # ---- interleave the graphs on the PE: g0's early passes fill the
        # window where g1's head is DMA-gated; drains are emitted so every
        # PSUM/SBUF buffer reuse is a forward dependency -----------------
        emit_dma(0)
        nc.sync.dma_start(W32_sb[:], W_d.ap().rearrange("(fb p) d -> p fb d", p=P))
        nc.sync.dma_start(cwh[:], cwh_d.ap())
        emit_dma(1)
        emit_head(0)
        emit_sst(0, 1, st[0]["p0"], cwh[:], st[0]["dinv"])
        emit_transp(0, 1)
        p01 = emit_pass(0, 1)
        emit_sst(0, 2, p01, st[0]["sigh"][:], st[0]["rdeg"])
        emit_head(1)
        emit_sst(1, 1, st[1]["p0"], cwh[:], st[1]["dinv"])
        emit_transp(1, 1)
        p11 = emit_pass(1, 1)
        emit_sst(1, 2, p11, st[1]["sigh"][:], st[1]["rdeg"])
        emit_transp(0, 2)
        p02 = emit_pass(0, 2)
        emit_sst(0, 3, p02, st[0]["sigh"][:], st[0]["rdeg"])
        emit_transp(1, 2)
        p12 = emit_pass(1, 2)
        emit_sst(1, 3, p12, st[1]["sigh"][:], st[1]["rdeg"])
        emit_transp(0, 3)
        p03 = emit_pass(0, 3)
        emit_out(0, p03)
        emit_transp(1, 3)
        p13 = emit_pass(1, 3)
        emit_out(1, p13)

    nc.compile()
    return nc


def _get_nc():
    if "nc" not in _COMPILED:
        _COMPILED["nc"] = _build()
    return _COMPILED["nc"]


def kernel(flows, W1, b1, W2, b2, W3, b3, _trace=False):
    from concourse.bass_utils import run_bass_kernel_spmd

    flows = np.asarray(flows, dtype=np.float32)
    W1 = np.asarray(W1, dtype=np.float32)
    W2 = np.asarray(W2, dtype=np.float32)
    W3 = np.asarray(W3, dtype=np.float32)
    b1 = np.asarray(b1, dtype=np.float32)
    b2 = np.asarray(b2, dtype=np.float32)
    b3 = np.asarray(b3, dtype=np.float32)

    nc = _get_nc()

    W123 = (W1 @ W2) @ W3                                   # [N, D] f32
    An8 = (flows - np.float32(0.5)).astype(ml_dtypes.float8_e4m3)
    At8 = np.ascontiguousarray(An8.transpose(0, 2, 1))
    W32 = (32.0 * W123).astype(ml_dtypes.float8_e4m3)
    cwh = (16.0 * W123.sum(axis=0, dtype=np.float64)).astype(np.float32)[:, None]

    in_maps = []
    for c in range(NCORES):
        in_maps.append({
            "An": An8[c * GPC:(c + 1) * GPC],
            "At": At8[c * GPC:(c + 1) * GPC],
            "W32": W32, "cwh": cwh,
        })

    res = run_bass_kernel_spmd(nc, in_maps, core_ids=list(range(NCORES)), trace=_trace)
    out = np.concatenate([res.results[c]["out"] for c in range(NCORES)], axis=0)
    out = np.ascontiguousarray(out.transpose(0, 2, 1)).astype(np.float32)

    if np.any(b1) or np.any(b2) or np.any(b3):
        # bias terms are rank-1: out += (^A^2 1) c1^T + (^A 1) c2^T + 1 b3^T
        deg = flows.sum(axis=1)
        dinv = np.where(deg > 0, 1.0 / np.sqrt(deg), 0.0).astype(np.float32)
        u1 = dinv * np.einsum("brc,br->bc", flows, dinv)
        u2 = dinv * np.einsum("brc,br->bc", flows, dinv * u1)
        c1 = (b1 @ W2) @ W3
        c2 = b2 @ W3
        out = out + u2[:, :, None] * c1 + u1[:, :, None] * c2 + b3

    if _trace:
        return out, res
    return out


# revision 46
# speedup vs baseline: 1.0160x; 1.0160x over previous
"""Trainium2 Bass kernel for nn_Encoder_Flows (3-layer dense GCN, linear).

The reference network has no nonlinearity, so per graph (A = flows [N,N]):
    out = ^A^3 (A @ W123) + bias-terms,   W123 = W1@W2@W3  (host-precomputed)
    ^A = D^-1/2 A^T D^-1/2,  deg[c] = sum_r A[r,c]
Bias terms are rank-1 (zero for the graded inputs; added on host if present).

A is centered on host: At~ = A - 0.5 (fp8 e4m3), so A = At~ + 0.5*ones.
The 0.5*ones*ones^T rank-1 part of every product collapses to a per-feature
constant (0.5 * colsum of the multiplicand) that is fused into the PSUM-drain
op; centering also kills the systematic fp8 quantization error that raw-A
fp8 matmuls would amplify by deg~1024.

All big matmuls keep A as the fp8 DoubleRow MOVING operand (2 elem/cycle;
the PE's stationary-load port is the bottleneck when A is stationary), with
small stationary tiles (W123 / msg k-tile pairs), so the whole chain is
computed feature-major:
  deg    : ones-stationary DR matmul over An chunks -> deg replicated [128,N]
  scales : sqrt (scalar) -> reciprocal/square (DVE, f16 replicated rows)
  pass0  : P0 = (32 W123)^T At~^T            (At~ chunks moving)
  pass k : Pk = msg_k^T At~                  (An chunks moving)
  drain  : msgT_{k+1} = (Pk + 0.5 sig_k[d]) * scale_rep  -- one DVE op per
           512-col PSUM piece, sig accumulated by the same op (accum_out)
  msgq   : 16 PE transposes per pass give the node-major fp8 stationary
           tiles for the next pass
  out    : (P3 + 0.5 sig3) * dinv/32, written feature-major [g, D, N]; the
           host transposes back to [g, N, D].
"""

import sys
from contextlib import ExitStack

import numpy as np

for _p in ("/opt/trn_rl_repo", "/opt/pypackages"):
    if _p not in sys.path:
        sys.path.append(_p)

import ml_dtypes

B, N, P = 16, 2048, 128
NB = N // P          # 16 row/col blocks
NQ = NB // 2         # 8 DoubleRow k-tile pairs
NCORES = 8
GPC = B // NCORES    # graphs per core
D = 128              # folded feature width (W123 columns)
CH = 512             # DMA / psum-piece column chunk
NCH = N // CH        # 4

_COMPILED = {}


def _build():
    import concourse.mybir as mybir
    import concourse.tile as tile
    from concourse import bacc
    from concourse.masks import make_identity

    f32 = mybir.dt.float32
    f16 = mybir.dt.float16
    fp8 = mybir.dt.float8e4
    DR = mybir.MatmulPerfMode.DoubleRow
    ADD = mybir.AluOpType.add
    MUL = mybir.AluOpType.mult
    X = mybir.AxisListType.X

    nc = bacc.Bacc("TRN2", target_bir_lowering=False)
    An_d = nc.declare_dram_parameter("An", [GPC, N, N], fp8, isOutput=False)
    At_d = nc.declare_dram_parameter("At", [GPC, N, N], fp8, isOutput=False)
    W_d = nc.declare_dram_parameter("W32", [N, D], fp8, isOutput=False)
    cwh_d = nc.declare_dram_parameter("cwh", [P, 1], f32, isOutput=False)
    out_d = nc.declare_dram_parameter("out", [GPC, D, N], f16, isOutput=True)

    with tile.TileContext(nc) as tc, ExitStack() as ctx:
        wpool = ctx.enter_context(tc.tile_pool(name="wpool", bufs=1))
        apool = ctx.enter_context(tc.tile_pool(name="apool", bufs=2))
        tpool = ctx.enter_context(tc.tile_pool(name="tpool", bufs=2))
        mtp = ctx.enter_context(tc.tile_pool(name="mtp", bufs=3))
        mqp = ctx.enter_context(tc.tile_pool(name="mqp", bufs=2))
        svec = ctx.enter_context(tc.tile_pool(name="svec", bufs=2))
        slim = ctx.enter_context(tc.tile_pool(name="slim", bufs=4))
        ogp = ctx.enter_context(tc.tile_pool(name="ogp", bufs=4))
        ppool = ctx.enter_context(tc.tile_pool(name="ppool", bufs=4, space="PSUM"))
        scr = ctx.enter_context(tc.tile_pool(name="scr", bufs=4, space="PSUM"))

        # --- replicated constants ---
        W32_sb = wpool.tile([P, NB, D], fp8)
        nc.sync.dma_start(W32_sb[:], W_d.ap().rearrange("(fb p) d -> p fb d", p=P))
        cwh = wpool.tile([P, 1], f32)
        nc.sync.dma_start(cwh[:], cwh_d.ap())
        onesf8 = wpool.tile([P, 2, P], fp8)
        nc.any.memset(onesf8[:], 1.0)
        idf16 = wpool.tile([P, P], f16)
        make_identity(nc, idf16[:])
        c1024 = wpool.tile([P, 1], f32)
        nc.any.memset(c1024[:], 1024.0)

        st = [{} for _ in range(GPC)]   # per-graph tiles

        # ---- DMA + deg + pass0, chunk-interleaved ----------------------
        def emit_dma(g):
            # one tile per DoubleRow k-pair, full 2048B rows per descriptor;
            # An first so deg/dinv complete while At streams. Issued for both
            # graphs before any output DMA so the in-order sync engine never
            # stalls g1's inputs behind g0's outputs.
            s = st[g]
            s["An"] = [apool.tile([P, 2, N], fp8, tag=f"An{q}", name=f"An{g}_{q}") for q in range(NQ)]
            s["At"] = [tpool.tile([P, 2, N], fp8, tag=f"At{q}", name=f"At{g}_{q}") for q in range(NQ)]
            for q in range(NQ):
                nc.sync.dma_start(
                    s["An"][q][:],
                    An_d.ap()[g].rearrange("(rb p) c -> p rb c", p=P)[:, 2 * q:2 * q + 2, :])
            for q in range(NQ):
                nc.sync.dma_start(
                    s["At"][q][:],
                    At_d.ap()[g].rearrange("(fb p) m -> p fb m", p=P)[:, 2 * q:2 * q + 2, :])

        def emit_head(g):
            s = st[g]
            # deg: ones-stationary, An moving, streamed in DMA-arrival order
            dchunks = [scr.tile([P, CH], f32, tag="scr", name=f"deg{g}_{ch}") for ch in range(NCH)]
            for ch in range(NCH):
                for h in range(2):
                    for q in range(NQ):
                        nc.tensor.matmul(
                            dchunks[ch][:, h * 256:(h + 1) * 256],
                            onesf8[:],
                            s["An"][q][:, :, ch * CH + h * 256:ch * CH + (h + 1) * 256],
                            start=(q == 0), stop=(q == NQ - 1), perf_mode=DR)

            # replicated scale rows, per-chunk pipeline
            dinv = svec.tile([P, N], f32, tag="dinv", name=f"dinv{g}")
            rdeg = svec.tile([P, N], f16, tag="rdeg", name=f"rdeg{g}")
            dinvh = svec.tile([P, N], f16, tag="dinvh", name=f"dinvh{g}")
            for ch in range(NCH):
                sl = slice(ch * CH, (ch + 1) * CH)
                sdeg = slim.tile([P, CH], f32, tag="sdeg", name=f"sdeg{g}_{ch}")
                nc.scalar.activation(
                    sdeg[:], dchunks[ch][:],
                    mybir.ActivationFunctionType.Sqrt, bias=c1024[:])
                nc.vector.reciprocal_approx_fast(dinv[:, sl], sdeg[:])
                with nc.allow_low_precision(reason="f16 scale rows; 5e-4 rel ok vs 2e-2 tol"):
                    nc.vector.tensor_tensor(rdeg[:, sl], dinv[:, sl], dinv[:, sl], MUL)
                    nc.scalar.mul(dinvh[:, sl], dinv[:, sl], 1.0 / 32.0)

            # pass0: W32 stationary, At moving (streamed) -> P0 = Y0^T
            s["p0"] = [ppool.tile([P, CH], f32, tag="pp", name=f"pp0_{g}_{ch}") for ch in range(NCH)]
            for ch in range(NCH):
                for h in range(2):
                    for q in range(NQ):
                        nc.tensor.matmul(
                            s["p0"][ch][:, h * 256:(h + 1) * 256],
                            W32_sb[:, 2 * q:2 * q + 2, :],
                            s["At"][q][:, :, ch * CH + h * 256:ch * CH + (h + 1) * 256],
                            start=(q == 0), stop=(q == NQ - 1), perf_mode=DR)
            s["dinv"], s["rdeg"], s["dinvh"] = dinv, rdeg, dinvh

        # drain psum pieces -> msgT (f16) + sig half-colsum (no PE work)
        def emit_sst(g, k, pieces, scol, srep):
            s = st[g]
            msgT = mtp.tile([P, N], f16, tag="msgT", name=f"msgT{g}_{k}")
            sig = slim.tile([P, NCH], f32, tag="sig", name=f"sg{g}_{k}")
            for ch in range(NCH):
                nc.vector.scalar_tensor_tensor(
                    msgT[:, ch * CH:(ch + 1) * CH], pieces[ch][:], scol,
                    srep[:, ch * CH:(ch + 1) * CH], ADD, MUL,
                    accum_out=sig[:, ch:ch + 1])
            sigh = slim.tile([P, 1], f32, tag="sigh", name=f"sgh{g}_{k}")
            nc.vector.tensor_reduce(sigh[:], sig[:], X, ADD)
            nc.scalar.mul(sigh[:], sigh[:], 0.5)
            s["msgT"], s["sigh"] = msgT, sigh

        # PE transposes of msgT -> node-major fp8 stationary for next pass
        # (batched 4 per PSUM bank; one 512-wide cast-copy per batch,
        # alternating scalar/DVE)
        def emit_transp(g, k):
            s = st[g]
            msgq = mqp.tile([P, NB, D], fp8, tag="msgq", name=f"msgq{g}_{k}")
            for grp in range(NB // 4):
                tp = scr.tile([P, 4, P], f16, tag="scr", name=f"tp{g}_{k}_{grp}")
                for j in range(4):
                    mb = grp * 4 + j
                    nc.tensor.transpose(tp[:, j, :], s["msgT"][:, mb * P:(mb + 1) * P], idf16[:])
                if grp % 2 == 0:
                    nc.scalar.copy(msgq[:, grp * 4:grp * 4 + 4, :], tp[:])
                else:
                    nc.vector.tensor_copy(msgq[:, grp * 4:grp * 4 + 4, :], tp[:])
            s["msgq"] = msgq

        # pass k chunk: msgq stationary, An moving -> P feature-major piece
        def emit_pass(g, k):
            s = st[g]
            msgq = s["msgq"]
            pieces = []
            for ch in range(NCH):
                pp = ppool.tile([P, CH], f32, tag="pp", name=f"pp{k}_{g}_{ch}")
                for h in range(2):
                    for q in range(NQ):
                        nc.tensor.matmul(
                            pp[:, h * 256:(h + 1) * 256],
                            msgq[:, 2 * q:2 * q + 2, :],
                            s["An"][q][:, :, ch * CH + h * 256:ch * CH + (h + 1) * 256],
                            start=(q == 0), stop=(q == NQ - 1), perf_mode=DR)
                pieces.append(pp)
            return pieces

        def emit_out(g, pieces):
            s = st[g]
            for ch in range(NCH):
                og = ogp.tile([P, CH], f16, tag="og", name=f"og{g}_{ch}")
                with nc.allow_low_precision(reason="f16 output; 5e-4 rel ok vs 2e-2 tol"):
                    nc.vector.scalar_tensor_tensor(
                        og[:], pieces[ch][:], s["sigh"][:],
                        s["dinvh"][:, ch * CH:(ch + 1) * CH], ADD, MUL)
                nc.sync.dma_start(out_d.ap()[g][:, ch * CH:(ch + 1) * CH], og[:])

        # ---- interleave the graphs on the PE: g0's early passes fill the
        # window where g1's head is DMA-gated; drains are emitted so every
        # PSUM/SBUF buffer reuse is a forward dependency -----------------
        emit_dma(0)
        emit_dma(1)
        emit_head(0)
        emit_sst(0, 1, st[0]["p0"], cwh[:], st[0]["dinv"])
        emit_transp(0, 1)
        p01 = emit_pass(0, 1)
        emit_sst(0, 2, p01, st[0]["sigh"][:], st[0]["rdeg"])
        emit_head(1)
        emit_sst(1, 1, st[1]["p0"], cwh[:], st[1]["dinv"])
        emit_transp(1, 1)
        p11 = emit_pass(1, 1)
        emit_sst(1, 2, p11, st[1]["sigh"][:], st[1]["rdeg"])
        emit_transp(0, 2)
        p02 = emit_pass(0, 2)
        emit_sst(0, 3, p02, st[0]["sigh"][:], st[0]["rdeg"])
        emit_transp(1, 2)
        p12 = emit_pass(1, 2)
        emit_sst(1, 3, p12, st[1]["sigh"][:], st[1]["rdeg"])
        emit_transp(0, 3)
        p03 = emit_pass(0, 3)
        emit_out(0, p03)
        emit_transp(1, 3)
        p13 = emit_pass(1, 3)
        emit_out(1, p13)

    nc.compile()
    return nc


def _get_nc():
    if "nc" not in _COMPILED:
        _COMPILED["nc"] = _build()
    return _COMPILED["nc"]


def kernel(flows, W1, b1, W2, b2, W3, b3, _trace=False):
    from concourse.bass_utils import run_bass_kernel_spmd

    flows = np.asarray(flows, dtype=np.float32)
    W1 = np.asarray(W1, dtype=np.float32)
    W2 = np.asarray(W2, dtype=np.float32)
    W3 = np.asarray(W3, dtype=np.float32)
    b1 = np.asarray(b1, dtype=np.float32)
    b2 = np.asarray(b2, dtype=np.float32)
    b3 = np.asarray(b3, dtype=np.float32)

    nc = _get_nc()

    W123 = (W1 @ W2) @ W3                                   # [N, D] f32
    An8 = (flows - np.float32(0.5)).astype(ml_dtypes.float8_e4m3)
    At8 = np.ascontiguousarray(An8.transpose(0, 2, 1))
    W32 = (32.0 * W123).astype(ml_dtypes.float8_e4m3)
    cwh = (16.0 * W123.sum(axis=0, dtype=np.float64)).astype(np.float32)[:, None]

    in_maps = []
    for c in range(NCORES):
        in_maps.append({
            "An": An8[c * GPC:(c + 1) * GPC],
            "At": At8[c * GPC:(c + 1) * GPC],
            "W32": W32, "cwh": cwh,
        })

    res = run_bass_kernel_spmd(nc, in_maps, core_ids=list(range(NCORES)), trace=_trace)
    out = np.concatenate([res.results[c]["out"] for c in range(NCORES)], axis=0)
    out = np.ascontiguousarray(out.transpose(0, 2, 1)).astype(np.float32)

    if np.any(b1) or np.any(b2) or np.any(b3):
        # bias terms are rank-1: out += (^A^2 1) c1^T + (^A 1) c2^T + 1 b3^T
        deg = flows.sum(axis=1)
        dinv = np.where(deg > 0, 1.0 / np.sqrt(deg), 0.0).astype(np.float32)
        u1 = dinv * np.einsum("brc,br->bc", flows, dinv)
        u2 = dinv * np.einsum("brc,br->bc", flows, dinv * u1)
        c1 = (b1 @ W2) @ W3
        c2 = b2 @ W3
        out = out + u2[:, :, None] * c1 + u1[:, :, None] * c2 + b3

    if _trace:
        return out, res
    return out
